# revision 25
# baseline (speedup 1.0000x reference)
"""Trainium2 Bass kernel for nn_CrossAdjacencyMatrix (gnn_message_passing).

Computes, for two independent sets (sr, tg):
    he, te, re = ent[h], ent[t], rel[r]                 (per-triple gathers)
    tv  = 1 - sum(|he + re - te|) * INV                 [N]
    A   = scatter(h,t){0.3*tv + 0.4*rel_w[r]}           [E,E] (positions unique)
    out = conf * imp * (0.3*pca + A) + I

Sharding: rows of the [E,E] adjacency are split into 8 blocks of 625 rows
(one per NeuronCore); triples are routed by head id to the owning core on
the host (sorted by head, padded to M slots per head row).  Embedding
tables are replicated.  Per core, per 128-row tile:
  - head embeddings arrive as a dense slice (no gather needed),
  - tail/rel embeddings are fetched with dma_gather from host-converted
    bf16 tables (chunked to 1024 descriptors per instruction to fit the
    SWDGE descriptor ring),
  - the VectorE computes the per-triple scores in a head-per-partition
    layout [128 heads, M slots],
  - local_scatter builds the sparse matrix A as dense f16 tiles directly
    in SBUF (per-partition tail indices; pad slots use idx=-1),
  - the dense pass streams a host-packed f16 [rows, 3, E] conf/imp/0.3*pca
    tensor (one full-row DMA per tile), computing conf*imp*(pca3 + A) in
    f16, storing f16 rows straight from the fused tile on the second
    HWDGE ring.
The unit diagonal and the f16->f32 upcast are applied on the host during
the unshard.  GPSIMD library phases (mlp dma_gather vs local_scatter) are
strictly separated so the Q7 iram is reloaded exactly once; the old
per-tile diagonal scatter-add (which forced ~20 Q7 library reloads, ~45us
each) is gone.

Measured on HW (reps-differencing): the kernel is ~90% bound by SWDGE
gather descriptor processing (~107k 256B descriptors/core at ~9ns/desc
aggregate); scores/scatter/dense overlap under it.  Descriptor count is
2/triple (te+re) and cannot legitimately shrink further; >1024-idx
dma_gather instructions crash (hard ring limit), multiple SWDGE queues
give <5%.
"""

import numpy as np

E = 5000
D = 128
R = 1000
NCORES = 8
RB = E // NCORES          # 625 rows per core
NT = 5                    # row tiles per core: 4x128 + 113
CH = 1250                 # local_scatter num_elems (< 2048)
NCH = E // CH
GW = 8                    # dma_gather M-slots per chunk (8*128 = 1024 descs)
INV = 1.0 / (3.0 * float(np.sqrt(D)))

_CACHE = {}


def _wrap16(flat_idx):
    """dma_gather index layout: flat list -> [128, len/16] int16, idx j at
    [j%16, j//16], replicated across the eight 16-partition core groups."""
    n = len(flat_idx)
    assert n % 16 == 0
    t = np.zeros((128, n // 16), np.int16)
    t[:16] = np.asarray(flat_idx, np.int16).reshape(n // 16, 16).T
    for b in range(1, 8):
        t[b * 16 : (b + 1) * 16] = t[:16]
    return t


def _prep_set(ent, rel, rw, h, t, r, M, gw=GW):
    """Per-core routed triple data for one set. Returns list of 8 dicts."""
    import ml_dtypes

    h = np.asarray(h, np.int64)
    t = np.asarray(t, np.int64)
    r = np.asarray(r, np.int64)
    rw = np.asarray(rw, np.float32)
    out = []
    for c in range(NCORES):
        sel = (h >= RB * c) & (h < RB * (c + 1))
        hl = (h[sel] - RB * c).astype(np.int64)
        tt = t[sel]
        rr = r[sel]
        order = np.argsort(hl, kind="stable")
        hl, tt, rr = hl[order], tt[order], rr[order]
        counts = np.bincount(hl, minlength=RB)
        starts = np.zeros(RB, np.int64)
        starts[1:] = np.cumsum(counts)[:-1]
        m_idx = np.arange(len(hl)) - starts[hl]
        assert counts.max() <= M, (counts.max(), M)

        NR = NT * 128  # 640 padded rows
        tid = np.zeros((NR, M), np.int64)   # pad -> row 0 (gathers garbage)
        rid = np.zeros((NR, M), np.int64)
        c0 = np.zeros((NR, M), np.float32)
        lsx = np.full((NR, M), -1, np.int64)  # local_scatter idx, pad -> -1
        tid[hl, m_idx] = tt
        rid[hl, m_idx] = rr
        c0[hl, m_idx] = 0.3 + 0.4 * rw[rr]
        lsx[hl, m_idx] = tt

        # gather indices, chunked so each dma_gather stays within the
        # SWDGE descriptor ring: chunk ci covers slots [gw*ci, gw*(ci+1))
        GC = (M + gw - 1) // gw
        gid_t = np.zeros((NT, 128, GC * gw * 8), np.int16)
        gid_r = np.zeros((NT, 128, GC * gw * 8), np.int16)
        for ti in range(NT):
            blk_t = tid[ti * 128 : (ti + 1) * 128]  # [128, M]
            blk_r = rid[ti * 128 : (ti + 1) * 128] + E  # rel rows sit at +E in tab
            for ci in range(GC):
                mlo, mhi = gw * ci, min(gw * (ci + 1), M)
                w = (mhi - mlo) * 128 // 16  # int16 cols used
                csl = slice(gw * 8 * ci, gw * 8 * ci + w)
                # flat order j = m_local*128 + p  ->  out[p, mlo+m_local, :]
                gid_t[ti, :, csl] = _wrap16(blk_t[:, mlo:mhi].T.reshape(-1))
                gid_r[ti, :, csl] = _wrap16(blk_r[:, mlo:mhi].T.reshape(-1))

        # local_scatter chunk indices: [NT, NCH, 128, M] int16
        lsx5 = lsx.reshape(NT, 128, M)
        lidx = np.full((NT, NCH, 128, M), -1, np.int16)
        for k in range(NCH):
            rel_k = lsx5 - k * CH
            ink = (lsx5 >= k * CH) & (lsx5 < (k + 1) * CH)
            lidx[:, k][ink] = rel_k[ink].astype(np.int16)

        # one bf16 table: ent rows [0,E), rel rows [E,E+R), per-core he
        # slices at [E+R, E+R+NR)
        tab = np.zeros((E + R + NR, D), ml_dtypes.bfloat16)
        tab[:E] = ent.astype(ml_dtypes.bfloat16)
        tab[E : E + R] = rel.astype(ml_dtypes.bfloat16)
        tab[E + R : E + R + RB] = ent[RB * c : RB * (c + 1)].astype(
            ml_dtypes.bfloat16
        )

        gix = np.concatenate([gid_t, gid_r], axis=2)
        lix = lidx.transpose(0, 2, 1, 3).reshape(NT, 128, NCH * M)
        out.append(
            {
                "tab": tab,
                "gix": np.ascontiguousarray(gix),
                "lix": np.ascontiguousarray(lix),
                "cm": np.ascontiguousarray(
                    c0.reshape(NT, 128, M).astype(np.float16)
                ),
            }
        )
    return out


def _patch_tile_tail():
    """This walrus build rejects instructions carrying more than one sync
    wait. Spread the Tile tail drain's sem waits across one nop each (the
    general _split_excess_waits pass then handles everything else)."""
    import concourse.tile as tile_mod
    import concourse.mybir as mybir
    from concourse.vector_clock import ScopedClock

    if getattr(tile_mod.TileContext, "_drain_patched", False):
        return

    def _patched(self, tick_clock, wait_clock):
        nc = self.nc
        nops = [nc.sync.nop(nofuse=True) for _ in range(8)]
        drain_inst = nc.sync.drain()
        wait_clock.add_sem_waits(
            drain_inst.ins, ScopedClock({None: tick_clock.global_clock})
        )
        waits = list(drain_inst.ins.sync_info.on_wait)
        if len(waits) > 1:
            drain_inst.ins.sync_info.on_wait = []
            for i, w in enumerate(waits):
                tgt = nops[i].ins if i < len(nops) else nc.sync.nop(nofuse=True).ins
                if tgt.sync_info is None:
                    tgt.sync_info = mybir.SyncInfo(on_wait=[], on_update=[])
                tgt.sync_info.on_wait = [w]
        nc.all_engine_barrier()
        assert self.sems is not None
        popped = nc._tile_sem_poison_stack.pop()
        assert popped is self._sem_poison
        nc.clear_and_free_semaphores(list(self.sems.allocated().values()))
        nc.all_engine_barrier()

    tile_mod.TileContext._drain_and_barrier = _patched
    tile_mod.TileContext._drain_patched = True


def _split_excess_waits(nc, limit=1):
    """Move excess sync waits onto same-engine InstNoOp instructions inserted
    immediately before the offender (same engine + program order => identical
    synchronization semantics)."""
    import concourse.mybir as mybir

    counter = [0]

    def fresh_nop(engine, wait):
        counter[0] += 1
        nop = mybir.InstNoOp(name=f"I-waitsplit-{counter[0]}", ins=[], outs=[])
        nop.engine = engine
        nop.sync_info = mybir.SyncInfo(on_wait=[wait], on_update=[])
        try:
            nc.register_instruction(nop, overwrite=True)
        except Exception:
            pass
        return nop

    for fn in nc.m.functions:
        for bb in fn.blocks:
            changed = False
            new_insts = []
            for inst in bb.instructions:
                si = getattr(inst, "sync_info", None)
                waits = list(si.on_wait) if si is not None and si.on_wait else []
                lim = 0 if inst.opcode == "Drain" else limit
                if len(waits) > lim:
                    excess = waits[: len(waits) - lim]
                    si.on_wait = waits[len(waits) - lim :]
                    for w in excess:
                        new_insts.append(fresh_nop(inst.engine, w))
                    changed = True
                new_insts.append(inst)
            if changed:
                bb.instructions = new_insts


def _finalize(nc):
    """Post-Tile passes for raw Bass: insert GPSIMD library loads (mlp for
    dma_gather, local_scatter lib), populate .instr for extended-ISA
    instructions, and split multi-wait instructions."""
    import concourse.mybir as mybir
    from concourse.library_config import all_libraries, standard
    import bass_rust

    mask = {}
    for lib in all_libraries:
        for it in lib.instructions:
            mask[it] = mask.get(it, 0) | (1 << lib.index)
    bass_rust.insert_library_loads(nc, mask, len(all_libraries), standard.index)
    mybir.codegen_inst_isa_subclasses(nc)
    _split_excess_waits(nc)


def _build_nc(M, finalize=True, reps=1, nq=1, gw=GW, fuse_score=False,
              scratch=32768, probe_no_gpsimd=False,
              probe_skip_score=False, probe_skip_dense=False,
              probe_no_scatter=False, probe_no_gather=False,
              probe_no_phase_dep=False, probe_orphan_gathers=False):
    from concourse import bass, mybir
    import concourse.tile as tile
    import bass_rust

    _patch_tile_tail()

    f32 = mybir.dt.float32
    bf16 = mybir.dt.bfloat16
    f16 = mybir.dt.float16
    i16 = mybir.dt.int16
    GC = (M + gw - 1) // gw
    G8 = GC * gw * 8
    nc = bass.Bass(num_swdge_queues=nq, dynamic_dma_scratch_size=scratch)
    T = {}
    for s in ("a", "b"):
        T[s] = dict(
            cip=nc.dram_tensor(f"cip_{s}", [RB, 3, E], f16, kind="ExternalInput"),
            tab=nc.dram_tensor(
                f"tab_{s}", [E + R + NT * 128, D], bf16, kind="ExternalInput"
            ),
            gix=nc.dram_tensor(f"gix_{s}", [NT, 128, 2 * G8], i16, kind="ExternalInput"),
            lix=nc.dram_tensor(f"lix_{s}", [NT, 128, NCH * M], i16, kind="ExternalInput"),
            cm=nc.dram_tensor(f"cm_{s}", [NT, 128, M], f16, kind="ExternalInput"),
            out=nc.dram_tensor(f"out_{s}", [RB, E], f16, kind="ExternalOutput"),
        )

    _nireg = {}

    def nireg(n):
        if n not in _nireg:
            _nireg[n] = nc.gpsimd.to_reg(n)
        return _nireg[n]

    with tile.TileContext(nc) as tc:
        with (
            tc.tile_pool(name="small", bufs=2) as ps,
            tc.tile_pool(name="vkeep", bufs=1) as pv,
            tc.tile_pool(name="gather", bufs=2) as pg,
            tc.tile_pool(name="amat", bufs=2) as pa,
            tc.tile_pool(name="dense", bufs=3) as pd,
        ):
          for rep in range(reps):
            gathers = {"a": [], "b": []}   # dma_gather insts (phase 1)
            scatters = {"a": [], "b": []}  # local_scatter insts (phase 2)
            v16 = {}
            # ---------- phase 1: gathers + scores ----------
            if probe_skip_score:
                for s in ("a", "b"):
                    for ti in range(NT):
                        vt = pv.tile([128, M], f16, tag=f"v16_{s}{ti}")
                        nc.vector.memset(vt[:], 0.25)
                        v16[(s, ti)] = vt
                        if probe_orphan_gathers:
                            ts = T[s]
                            gixt = ps.tile([128, 2 * G8], i16, tag="gixt")
                            nc.sync.dma_start(out=gixt[:], in_=ts["gix"][ti])
                            te = pg.tile([128, M, D], bf16, tag="te")
                            re = pg.tile([128, M, D], bf16, tag="re")
                            for ci in range(GC):
                                mlo, mhi = gw * ci, min(gw * (ci + 1), M)
                                nic = (mhi - mlo) * 128
                                isl = slice(gw * 8 * ci, gw * 8 * ci + nic // 16)
                                gathers[s].append(nc.gpsimd.dma_gather(
                                    te[:, mlo:mhi, :], ts["tab"][:],
                                    gixt[:, isl], nic, nireg(nic), D,
                                ))
                                isl2 = slice(
                                    G8 + gw * 8 * ci, G8 + gw * 8 * ci + nic // 16
                                )
                                gathers[s].append(nc.gpsimd.dma_gather(
                                    re[:, mlo:mhi, :], ts["tab"][:],
                                    gixt[:, isl2], nic, nireg(nic), D,
                                ))
            else:
                for s in ("a", "b"):
                    ts = T[s]
                    for ti in range(NT):
                        he = ps.tile([128, D], bf16, tag="he")
                        base = E + R + 128 * ti
                        nc.sync.dma_start(
                            out=he[:], in_=ts["tab"][base : base + 128]
                        )
                        gixt = ps.tile([128, 2 * G8], i16, tag="gixt")
                        nc.sync.dma_start(out=gixt[:], in_=ts["gix"][ti])
                        git = gixt[:, 0:G8]
                        gir = gixt[:, G8 : 2 * G8]
                        cmt = ps.tile([128, M], f16, tag="cmt")
                        nc.sync.dma_start(out=cmt[:], in_=ts["cm"][ti])

                        te = pg.tile([128, M, D], bf16, tag="te")
                        re = pg.tile([128, M, D], bf16, tag="re")
                        if probe_no_gpsimd or probe_no_gather:
                            nc.vector.memset(te[:], 0.25)
                            nc.vector.memset(re[:], 0.5)
                        else:
                            for ci in range(GC):
                                mlo, mhi = gw * ci, min(gw * (ci + 1), M)
                                nic = (mhi - mlo) * 128
                                isl = slice(gw * 8 * ci, gw * 8 * ci + nic // 16)
                                g1 = nc.gpsimd.dma_gather(
                                    te[:, mlo:mhi, :], ts["tab"][:], git[:, isl],
                                    nic, nireg(nic), D,
                                    queue_num=(2 * ci) % nq,
                                )
                                g2 = nc.gpsimd.dma_gather(
                                    re[:, mlo:mhi, :], ts["tab"][:], gir[:, isl],
                                    nic, nireg(nic), D,
                                    queue_num=(2 * ci + 1) % nq,
                                )
                                gathers[s] += [g1, g2]

                        red = ps.tile([128, M], f32, tag="red")
                        # per-gather-chunk score ops: each chunk's compute
                        # starts as soon as its own gathers land, overlapping
                        # the remaining chunks' DMA
                        crng = [(0, M)] if fuse_score else [
                            (gw * ci, min(gw * (ci + 1), M)) for ci in range(GC)
                        ]
                        for mlo, mhi in crng:
                            mw = mhi - mlo
                            heb = he[:].unsqueeze(1).to_broadcast([128, mw, D])
                            nc.vector.tensor_tensor(
                                out=re[:, mlo:mhi, :], in0=re[:, mlo:mhi, :],
                                in1=heb, op=mybir.AluOpType.add,
                            )
                            nc.vector.tensor_tensor(
                                out=te[:, mlo:mhi, :], in0=re[:, mlo:mhi, :],
                                in1=te[:, mlo:mhi, :],
                                op=mybir.AluOpType.subtract,
                            )
                            nc.vector.tensor_reduce(
                                out=red[:, mlo:mhi],
                                in_=te[:, mlo:mhi, :],
                                axis=mybir.AxisListType.X,
                                op=mybir.AluOpType.add,
                                apply_absolute_value=True,
                            )
                        # v = c0 - 0.3*INV*red  (single fused op; tiny free dim)
                        vt = pv.tile([128, M], f16, tag=f"v16_{s}{ti}")
                        nc.vector.scalar_tensor_tensor(
                            out=vt[:], in0=red[:], scalar=-0.3 * INV, in1=cmt[:],
                            op0=mybir.AluOpType.mult, op1=mybir.AluOpType.add,
                        )
                        v16[(s, ti)] = vt

            # ---------- phases 2+3: scatter into A, dense combine ----------
            if not probe_skip_dense:
                for s in ("a", "b"):
                    ts = T[s]
                    for ti in range(NT):
                        nrows = RB - 128 * ti if ti == NT - 1 else 128
                        rsl = slice(128 * ti, 128 * ti + nrows)

                        lixt = ps.tile([128, NCH * M], i16, tag="lixt")
                        nc.sync.dma_start(out=lixt[:], in_=ts["lix"][ti])
                        amat = pa.tile([128, E], f16, tag="A")
                        if probe_no_gpsimd or probe_no_scatter:
                            nc.vector.memset(amat[:], 0.0)
                        else:
                            for k in range(NCH):
                                sc = nc.gpsimd.local_scatter(
                                    amat[:, CH * k : CH * (k + 1)],
                                    v16[(s, ti)][:],
                                    lixt[:, M * k : M * (k + 1)],
                                    128,
                                    CH,
                                    M,
                                )
                                scatters[s].append(sc)

                        cip = pd.tile([128, 3, E], f16, tag="cip")
                        nc.sync.dma_start(out=cip[:nrows], in_=ts["cip"][rsl])
                        cf = cip[:, 0, :]
                        im = cip[:, 1, :]
                        p3 = cip[:, 2, :]  # host-packed 0.3*pca
                        nc.vector.tensor_tensor(
                            out=p3[:nrows], in0=p3[:nrows], in1=amat[:nrows, :],
                            op=mybir.AluOpType.add,
                        )
                        nc.vector.tensor_tensor(
                            out=cf[:nrows], in0=cf[:nrows], in1=im[:nrows],
                            op=mybir.AluOpType.mult,
                        )
                        nc.vector.tensor_tensor(
                            out=p3[:nrows], in0=cf[:nrows], in1=p3[:nrows],
                            op=mybir.AluOpType.mult,
                        )
                        nc.scalar.dma_start(out=ts["out"][rsl], in_=p3[:nrows])

            # phase separation: every local_scatter after every dma_gather so
            # the GPSIMD library (mlp <-> local_scatter) switches once per rep
            if not probe_no_phase_dep:
                for s2 in ("a", "b"):
                    for sc in scatters[s2]:
                        for g in gathers["a"] + gathers["b"]:
                            bass_rust.add_dep_helper(
                                sc.ins, g.ins, False, "library phase separation"
                            )

    if finalize:
        _finalize(nc)
    return nc


class _Runner:
    """Compiles the SPMD bass program through PJRT once and keeps the jitted
    callable for repeated (timed) execution. Mirrors
    concourse.bass2jax.run_bass_via_pjrt, minus output-donation (the kernel
    writes every output element, so uninitialized result buffers are fine)."""

    def __init__(self, nc):
        import jax
        from jax.sharding import Mesh, PartitionSpec
        from jax.experimental.shard_map import shard_map
        from concourse import bass2jax
        import concourse.mybir as _mybir

        bass2jax.install_neuronx_cc_hook()
        self.jax = jax
        partition_name = (
            nc.partition_id_tensor.name if nc.partition_id_tensor else None
        )
        in_names, out_names, out_avals = [], [], []
        for alloc in nc.m.functions[0].allocations:
            if not isinstance(alloc, _mybir.MemoryLocationSet):
                continue
            name = alloc.memorylocations[0].name
            if alloc.kind == "ExternalInput":
                if name != partition_name:
                    in_names.append(name)
            elif alloc.kind == "ExternalOutput":
                out_names.append(name)
                out_avals.append(
                    jax.core.ShapedArray(
                        tuple(alloc.tensor_shape), _mybir.dt.np(alloc.dtype)
                    )
                )
        self.in_names, self.out_names, self.out_avals = in_names, out_names, out_avals

        bind_in_names = tuple(in_names) + (
            (partition_name,) if partition_name else ()
        )

        def _body(*args):
            operands = list(args)
            if partition_name is not None:
                operands.append(bass2jax.partition_id_tensor())
            outs = bass2jax._bass_exec_p.bind(
                *operands,
                out_avals=tuple(out_avals),
                in_names=bind_in_names,
                out_names=tuple(out_names),
                lowering_input_output_aliases=(),
                sim_require_finite=True,
                sim_require_nnan=True,
                nc=nc,
            )
            return tuple(outs)

        devices = jax.devices()[:NCORES]
        self.mesh = Mesh(np.asarray(devices), ("core",))
        in_specs = (PartitionSpec("core"),) * len(in_names)
        out_specs = (PartitionSpec("core"),) * len(out_names)
        self.fn = jax.jit(
            shard_map(
                _body,
                mesh=self.mesh,
                in_specs=in_specs,
                out_specs=out_specs,
                check_rep=False,
            ),
            keep_unused=True,
        )

    def concat_inputs(self, in_maps):
        return [
            np.concatenate([np.asarray(in_maps[c][n]) for c in range(NCORES)], axis=0)
            for n in self.in_names
        ]

    def run(self, concat_in):
        return self.fn(*concat_in)

    def split_outputs(self, out_arrs):
        res = []
        for c in range(NCORES):
            res.append(
                {
                    n: np.asarray(out_arrs[i]).reshape(
                        NCORES, *self.out_avals[i].shape
                    )[c]
                    for i, n in enumerate(self.out_names)
                }
            )
        return res


def _get_runner(M):
    key = ("runner", M)
    if key not in _CACHE:
        nc = _build_nc(M)
        _CACHE[key] = _Runner(nc)
    return _CACHE[key]


def _host_prep(inputs, gw=GW):
    import ml_dtypes

    ent_a = np.asarray(inputs["ent_emb_sr"], np.float32)
    ent_b = np.asarray(inputs["ent_emb_tg"], np.float32)
    rel_a = np.asarray(inputs["rel_emb_sr"], np.float32)
    rel_b = np.asarray(inputs["rel_emb_tg"], np.float32)

    Ms = []
    for sfx in ("sr", "tg"):
        h = np.asarray(inputs[f"head_{sfx}"], np.int64)
        Ms.append(
            max(
                np.bincount(
                    h[(h >= RB * c) & (h < RB * (c + 1))] - RB * c, minlength=RB
                ).max()
                for c in range(NCORES)
            )
        )
    M = int(max(Ms))
    M += M & 1  # local_scatter needs num_idxs % 2 == 0

    in_maps = [dict() for _ in range(NCORES)]
    for s in ("a", "b"):
        sfx = "sr" if s == "a" else "tg"
        ent = ent_a if s == "a" else ent_b
        rel = rel_a if s == "a" else rel_b
        routed = _prep_set(
            ent,
            rel,
            inputs[f"relation_w_{sfx}"],
            inputs[f"head_{sfx}"],
            inputs[f"tail_{sfx}"],
            inputs[f"rel_{sfx}"],
            M,
            gw,
        )
        conf = np.asarray(inputs[f"conf_{sfx}"], np.float32)
        imp = np.asarray(inputs[f"imp_{sfx}"], np.float32)
        pca3 = np.asarray(inputs[f"pca_{sfx}"], np.float32) * np.float32(0.3)
        for c in range(NCORES):
            m = in_maps[c]
            rs = slice(RB * c, RB * (c + 1))
            m[f"cip_{s}"] = np.ascontiguousarray(
                np.stack([conf[rs], imp[rs], pca3[rs]], axis=1)
            ).astype(np.float16)
            m[f"tab_{s}"] = routed[c]["tab"]
            m[f"gix_{s}"] = routed[c]["gix"]
            m[f"lix_{s}"] = routed[c]["lix"]
            m[f"cm_{s}"] = routed[c]["cm"]
    return M, in_maps


def kernel(**inputs):
    M, in_maps = _host_prep(inputs)
    try:
        from concourse._compat import axon_active

        use_pjrt = axon_active()
    except Exception:
        use_pjrt = True
    if use_pjrt:
        runner = _get_runner(M)
        concat_in = runner.concat_inputs(in_maps)
        out_arrs = runner.run(concat_in)
        res = runner.split_outputs(out_arrs)
    else:
        from concourse.bass_utils import run_bass_kernel_spmd

        key = ("nc", M)
        if key not in _CACHE:
            _CACHE[key] = _build_nc(M)
        res = run_bass_kernel_spmd(_CACHE[key], in_maps, list(range(NCORES))).results
    idx = np.arange(E)
    adj_sr = np.concatenate(
        [res[c]["out_a"] for c in range(NCORES)], axis=0
    ).astype(np.float32)
    adj_tg = np.concatenate(
        [res[c]["out_b"] for c in range(NCORES)], axis=0
    ).astype(np.float32)
    adj_sr[idx, idx] += 1.0
    adj_tg[idx, idx] += 1.0
    return adj_sr, adj_tg


# revision 26
# speedup vs baseline: 1.5824x; 1.5824x over previous
"""Trainium2 Bass kernel for nn_CrossAdjacencyMatrix (gnn_message_passing).

Computes, for two independent sets (sr, tg):
    he, te, re = ent[h], ent[t], rel[r]                 (per-triple gathers)
    tv  = 1 - sum(|he + re - te|) * INV                 [N]
    A   = scatter(h,t){0.3*tv + 0.4*rel_w[r]}           [E,E] (positions unique)
    out = conf * imp * (0.3*pca + A) + I

Sharding: rows of the [E,E] adjacency are split into 8 blocks of 625 rows
(one per NeuronCore); triples are routed by head id to the owning core on
the host (sorted by head, padded to M slots per head row).  Embedding
tables are replicated.  Per core, per 128-row tile:
  - head embeddings arrive as a dense slice (no gather needed),
  - tail/rel embeddings are fetched with dma_gather from host-converted
    bf16 tables (chunked to 1024 descriptors per instruction to fit the
    SWDGE descriptor ring),
  - the VectorE computes the per-triple scores in a head-per-partition
    layout [128 heads, M slots],
  - local_scatter builds the sparse matrix A as dense f16 tiles directly
    in SBUF (per-partition tail indices; pad slots use idx=-1),
  - the dense pass streams a host-packed f16 [rows, 3, E] conf/imp/0.3*pca
    tensor (one full-row DMA per tile), computing conf*imp*(pca3 + A) in
    f16, storing f16 rows straight from the fused tile on the second
    HWDGE ring.
The unit diagonal and the f16->f32 upcast are applied on the host during
the unshard.  GPSIMD library phases (mlp dma_gather vs local_scatter) are
strictly separated so the Q7 iram is reloaded exactly once; the old
per-tile diagonal scatter-add (which forced ~20 Q7 library reloads, ~45us
each) is gone.

Measured on HW (reps-differencing): the kernel is ~90% bound by SWDGE
gather descriptor processing (~107k 256B descriptors/core at ~9ns/desc
aggregate); scores/scatter/dense overlap under it.  Descriptor count is
2/triple (te+re) and cannot legitimately shrink further; >1024-idx
dma_gather instructions crash (hard ring limit), multiple SWDGE queues
give <5%.
"""

import numpy as np

E = 5000
D = 128
R = 1000
NCORES = 8
RB = E // NCORES          # 625 rows per core
NT = 5                    # row tiles per core: 4x128 + 113
CH = 1250                 # local_scatter num_elems (< 2048)
NCH = E // CH
GW = 8                    # dma_gather M-slots per chunk (8*128 = 1024 descs)
INV = 1.0 / (3.0 * float(np.sqrt(D)))

_CACHE = {}


def _wrap16(flat_idx):
    """dma_gather index layout: flat list -> [128, len/16] int16, idx j at
    [j%16, j//16], replicated across the eight 16-partition core groups."""
    n = len(flat_idx)
    assert n % 16 == 0
    t = np.zeros((128, n // 16), np.int16)
    t[:16] = np.asarray(flat_idx, np.int16).reshape(n // 16, 16).T
    for b in range(1, 8):
        t[b * 16 : (b + 1) * 16] = t[:16]
    return t


def _prep_set(ent, rel, rw, h, t, r, M, gw=GW):
    """Per-core routed triple data for one set. Returns list of 8 dicts."""
    import ml_dtypes

    h = np.asarray(h, np.int64)
    t = np.asarray(t, np.int64)
    r = np.asarray(r, np.int64)
    rw = np.asarray(rw, np.float32)
    out = []
    for c in range(NCORES):
        sel = (h >= RB * c) & (h < RB * (c + 1))
        hl = (h[sel] - RB * c).astype(np.int64)
        tt = t[sel]
        rr = r[sel]
        order = np.argsort(hl, kind="stable")
        hl, tt, rr = hl[order], tt[order], rr[order]
        counts = np.bincount(hl, minlength=RB)
        starts = np.zeros(RB, np.int64)
        starts[1:] = np.cumsum(counts)[:-1]
        m_idx = np.arange(len(hl)) - starts[hl]
        assert counts.max() <= M, (counts.max(), M)

        NR = NT * 128  # 640 padded rows
        tid = np.zeros((NR, M), np.int64)   # pad -> row 0 (gathers garbage)
        rid = np.zeros((NR, M), np.int64)
        c0 = np.zeros((NR, M), np.float32)
        lsx = np.full((NR, M), -1, np.int64)  # local_scatter idx, pad -> -1
        tid[hl, m_idx] = tt
        rid[hl, m_idx] = rr
        c0[hl, m_idx] = 0.3 + 0.4 * rw[rr]
        lsx[hl, m_idx] = tt

        # gather indices, chunked so each dma_gather stays within the
        # SWDGE descriptor ring: chunk ci covers slots [gw*ci, gw*(ci+1))
        GC = (M + gw - 1) // gw
        gid_t = np.zeros((NT, 128, GC * gw * 8), np.int16)
        gid_r = np.zeros((NT, 128, GC * gw * 8), np.int16)
        for ti in range(NT):
            blk_t = tid[ti * 128 : (ti + 1) * 128]  # [128, M]
            blk_r = rid[ti * 128 : (ti + 1) * 128] + E  # rel rows sit at +E in tab
            for ci in range(GC):
                mlo, mhi = gw * ci, min(gw * (ci + 1), M)
                w = (mhi - mlo) * 128 // 16  # int16 cols used
                csl = slice(gw * 8 * ci, gw * 8 * ci + w)
                # flat order j = m_local*128 + p  ->  out[p, mlo+m_local, :]
                gid_t[ti, :, csl] = _wrap16(blk_t[:, mlo:mhi].T.reshape(-1))
                gid_r[ti, :, csl] = _wrap16(blk_r[:, mlo:mhi].T.reshape(-1))

        # local_scatter chunk indices: [NT, NCH, 128, M] int16
        lsx5 = lsx.reshape(NT, 128, M)
        lidx = np.full((NT, NCH, 128, M), -1, np.int16)
        for k in range(NCH):
            rel_k = lsx5 - k * CH
            ink = (lsx5 >= k * CH) & (lsx5 < (k + 1) * CH)
            lidx[:, k][ink] = rel_k[ink].astype(np.int16)

        # one bf16 table: ent rows [0,E), rel rows [E,E+R), per-core he
        # slices at [E+R, E+R+NR)
        tab = np.zeros((E + R + NR, D), ml_dtypes.bfloat16)
        tab[:E] = ent.astype(ml_dtypes.bfloat16)
        tab[E : E + R] = rel.astype(ml_dtypes.bfloat16)
        tab[E + R : E + R + RB] = ent[RB * c : RB * (c + 1)].astype(
            ml_dtypes.bfloat16
        )

        gix = np.concatenate([gid_t, gid_r], axis=2)
        lix = lidx.transpose(0, 2, 1, 3).reshape(NT, 128, NCH * M)
        out.append(
            {
                "tab": tab,
                "gix": np.ascontiguousarray(gix),
                "lix": np.ascontiguousarray(lix),
                "cm": np.ascontiguousarray(
                    c0.reshape(NT, 128, M).astype(np.float16)
                ),
            }
        )
    return out


def _patch_tile_tail():
    """This walrus build rejects instructions carrying more than one sync
    wait. Spread the Tile tail drain's sem waits across one nop each (the
    general _split_excess_waits pass then handles everything else)."""
    import concourse.tile as tile_mod
    import concourse.mybir as mybir
    from concourse.vector_clock import ScopedClock

    if getattr(tile_mod.TileContext, "_drain_patched", False):
        return

    def _patched(self, tick_clock, wait_clock):
        nc = self.nc
        nops = [nc.sync.nop(nofuse=True) for _ in range(8)]
        drain_inst = nc.sync.drain()
        wait_clock.add_sem_waits(
            drain_inst.ins, ScopedClock({None: tick_clock.global_clock})
        )
        waits = list(drain_inst.ins.sync_info.on_wait)
        if len(waits) > 1:
            drain_inst.ins.sync_info.on_wait = []
            for i, w in enumerate(waits):
                tgt = nops[i].ins if i < len(nops) else nc.sync.nop(nofuse=True).ins
                if tgt.sync_info is None:
                    tgt.sync_info = mybir.SyncInfo(on_wait=[], on_update=[])
                tgt.sync_info.on_wait = [w]
        nc.all_engine_barrier()
        assert self.sems is not None
        popped = nc._tile_sem_poison_stack.pop()
        assert popped is self._sem_poison
        nc.clear_and_free_semaphores(list(self.sems.allocated().values()))
        nc.all_engine_barrier()

    tile_mod.TileContext._drain_and_barrier = _patched
    tile_mod.TileContext._drain_patched = True


def _split_excess_waits(nc, limit=1):
    """Move excess sync waits onto same-engine InstNoOp instructions inserted
    immediately before the offender (same engine + program order => identical
    synchronization semantics)."""
    import concourse.mybir as mybir

    counter = [0]

    def fresh_nop(engine, wait):
        counter[0] += 1
        nop = mybir.InstNoOp(name=f"I-waitsplit-{counter[0]}", ins=[], outs=[])
        nop.engine = engine
        nop.sync_info = mybir.SyncInfo(on_wait=[wait], on_update=[])
        try:
            nc.register_instruction(nop, overwrite=True)
        except Exception:
            pass
        return nop

    for fn in nc.m.functions:
        for bb in fn.blocks:
            changed = False
            new_insts = []
            for inst in bb.instructions:
                si = getattr(inst, "sync_info", None)
                waits = list(si.on_wait) if si is not None and si.on_wait else []
                lim = 0 if inst.opcode == "Drain" else limit
                if len(waits) > lim:
                    excess = waits[: len(waits) - lim]
                    si.on_wait = waits[len(waits) - lim :]
                    for w in excess:
                        new_insts.append(fresh_nop(inst.engine, w))
                    changed = True
                new_insts.append(inst)
            if changed:
                bb.instructions = new_insts


def _finalize(nc):
    """Post-Tile passes for raw Bass: insert GPSIMD library loads (mlp for
    dma_gather, local_scatter lib), populate .instr for extended-ISA
    instructions, and split multi-wait instructions."""
    import concourse.mybir as mybir
    from concourse.library_config import all_libraries, standard
    import bass_rust

    mask = {}
    for lib in all_libraries:
        for it in lib.instructions:
            mask[it] = mask.get(it, 0) | (1 << lib.index)
    bass_rust.insert_library_loads(nc, mask, len(all_libraries), standard.index)
    mybir.codegen_inst_isa_subclasses(nc)
    _split_excess_waits(nc)


def _build_nc(M, finalize=True, reps=1, nq=2, gw=GW, fuse_score=False,
              scratch=32768, probe_no_gpsimd=False,
              probe_skip_score=False, probe_skip_dense=False,
              probe_no_scatter=False, probe_no_gather=False,
              probe_no_phase_dep=False, probe_orphan_gathers=False):
    from concourse import bass, mybir
    import concourse.tile as tile
    import bass_rust

    _patch_tile_tail()

    f32 = mybir.dt.float32
    bf16 = mybir.dt.bfloat16
    f16 = mybir.dt.float16
    i16 = mybir.dt.int16
    GC = (M + gw - 1) // gw
    G8 = GC * gw * 8
    nc = bass.Bass(num_swdge_queues=nq, dynamic_dma_scratch_size=scratch)
    T = {}
    for s in ("a", "b"):
        T[s] = dict(
            cip=nc.dram_tensor(f"cip_{s}", [RB, 3, E], f16, kind="ExternalInput"),
            tab=nc.dram_tensor(
                f"tab_{s}", [E + R + NT * 128, D], bf16, kind="ExternalInput"
            ),
            gix=nc.dram_tensor(f"gix_{s}", [NT, 128, 2 * G8], i16, kind="ExternalInput"),
            lix=nc.dram_tensor(f"lix_{s}", [NT, 128, NCH * M], i16, kind="ExternalInput"),
            cm=nc.dram_tensor(f"cm_{s}", [NT, 128, M], f16, kind="ExternalInput"),
            out=nc.dram_tensor(f"out_{s}", [RB, E], f16, kind="ExternalOutput"),
        )

    _nireg = {}

    def nireg(n):
        if n not in _nireg:
            _nireg[n] = nc.gpsimd.to_reg(n)
        return _nireg[n]

    with tile.TileContext(nc) as tc:
        with (
            tc.tile_pool(name="small", bufs=2) as ps,
            tc.tile_pool(name="vkeep", bufs=1) as pv,
            tc.tile_pool(name="gather", bufs=2) as pg,
            tc.tile_pool(name="amat", bufs=2) as pa,
            tc.tile_pool(name="dense", bufs=3) as pd,
        ):
          for rep in range(reps):
            gathers = {"a": [], "b": []}   # dma_gather insts (phase 1)
            scatters = {"a": [], "b": []}  # local_scatter insts (phase 2)
            v16 = {}
            # ---------- phase 1: gathers + scores ----------
            if probe_skip_score:
                for s in ("a", "b"):
                    for ti in range(NT):
                        vt = pv.tile([128, M], f16, tag=f"v16_{s}{ti}")
                        nc.vector.memset(vt[:], 0.25)
                        v16[(s, ti)] = vt
                        if probe_orphan_gathers:
                            ts = T[s]
                            gixt = ps.tile([128, 2 * G8], i16, tag="gixt")
                            nc.sync.dma_start(out=gixt[:], in_=ts["gix"][ti])
                            te = pg.tile([128, M, D], bf16, tag="te")
                            re = pg.tile([128, M, D], bf16, tag="re")
                            for ci in range(GC):
                                mlo, mhi = gw * ci, min(gw * (ci + 1), M)
                                nic = (mhi - mlo) * 128
                                isl = slice(gw * 8 * ci, gw * 8 * ci + nic // 16)
                                gathers[s].append(nc.gpsimd.dma_gather(
                                    te[:, mlo:mhi, :], ts["tab"][:],
                                    gixt[:, isl], nic, nireg(nic), D,
                                ))
                                isl2 = slice(
                                    G8 + gw * 8 * ci, G8 + gw * 8 * ci + nic // 16
                                )
                                gathers[s].append(nc.gpsimd.dma_gather(
                                    re[:, mlo:mhi, :], ts["tab"][:],
                                    gixt[:, isl2], nic, nireg(nic), D,
                                ))
            else:
                for s in ("a", "b"):
                    ts = T[s]
                    for ti in range(NT):
                        he = ps.tile([128, D], bf16, tag="he")
                        base = E + R + 128 * ti
                        nc.sync.dma_start(
                            out=he[:], in_=ts["tab"][base : base + 128]
                        )
                        gixt = ps.tile([128, 2 * G8], i16, tag="gixt")
                        nc.sync.dma_start(out=gixt[:], in_=ts["gix"][ti])
                        git = gixt[:, 0:G8]
                        gir = gixt[:, G8 : 2 * G8]
                        cmt = ps.tile([128, M], f16, tag="cmt")
                        nc.sync.dma_start(out=cmt[:], in_=ts["cm"][ti])

                        te = pg.tile([128, M, D], bf16, tag="te")
                        re = pg.tile([128, M, D], bf16, tag="re")
                        if probe_no_gpsimd or probe_no_gather:
                            nc.vector.memset(te[:], 0.25)
                            nc.vector.memset(re[:], 0.5)
                        else:
                            for ci in range(GC):
                                mlo, mhi = gw * ci, min(gw * (ci + 1), M)
                                nic = (mhi - mlo) * 128
                                isl = slice(gw * 8 * ci, gw * 8 * ci + nic // 16)
                                g1 = nc.gpsimd.dma_gather(
                                    te[:, mlo:mhi, :], ts["tab"][:], git[:, isl],
                                    nic, nireg(nic), D,
                                    queue_num=(2 * ci) % nq,
                                )
                                g2 = nc.gpsimd.dma_gather(
                                    re[:, mlo:mhi, :], ts["tab"][:], gir[:, isl],
                                    nic, nireg(nic), D,
                                    queue_num=(2 * ci + 1) % nq,
                                )
                                gathers[s] += [g1, g2]

                        red = ps.tile([128, M], f32, tag="red")
                        # per-gather-chunk score ops: each chunk's compute
                        # starts as soon as its own gathers land, overlapping
                        # the remaining chunks' DMA
                        crng = [(0, M)] if fuse_score else [
                            (gw * ci, min(gw * (ci + 1), M)) for ci in range(GC)
                        ]
                        for mlo, mhi in crng:
                            mw = mhi - mlo
                            heb = he[:].unsqueeze(1).to_broadcast([128, mw, D])
                            nc.vector.tensor_tensor(
                                out=re[:, mlo:mhi, :], in0=re[:, mlo:mhi, :],
                                in1=heb, op=mybir.AluOpType.add,
                            )
                            nc.vector.tensor_tensor(
                                out=te[:, mlo:mhi, :], in0=re[:, mlo:mhi, :],
                                in1=te[:, mlo:mhi, :],
                                op=mybir.AluOpType.subtract,
                            )
                            nc.vector.tensor_reduce(
                                out=red[:, mlo:mhi],
                                in_=te[:, mlo:mhi, :],
                                axis=mybir.AxisListType.X,
                                op=mybir.AluOpType.add,
                                apply_absolute_value=True,
                            )
                        # v = c0 - 0.3*INV*red  (single fused op; tiny free dim)
                        vt = pv.tile([128, M], f16, tag=f"v16_{s}{ti}")
                        nc.vector.scalar_tensor_tensor(
                            out=vt[:], in0=red[:], scalar=-0.3 * INV, in1=cmt[:],
                            op0=mybir.AluOpType.mult, op1=mybir.AluOpType.add,
                        )
                        v16[(s, ti)] = vt

            # ---------- phases 2+3: scatter into A, dense combine ----------
            if not probe_skip_dense:
                for s in ("a", "b"):
                    ts = T[s]
                    for ti in range(NT):
                        nrows = RB - 128 * ti if ti == NT - 1 else 128
                        rsl = slice(128 * ti, 128 * ti + nrows)

                        lixt = ps.tile([128, NCH * M], i16, tag="lixt")
                        nc.sync.dma_start(out=lixt[:], in_=ts["lix"][ti])
                        amat = pa.tile([128, E], f16, tag="A")
                        if probe_no_gpsimd or probe_no_scatter:
                            nc.vector.memset(amat[:], 0.0)
                        else:
                            for k in range(NCH):
                                sc = nc.gpsimd.local_scatter(
                                    amat[:, CH * k : CH * (k + 1)],
                                    v16[(s, ti)][:],
                                    lixt[:, M * k : M * (k + 1)],
                                    128,
                                    CH,
                                    M,
                                )
                                scatters[s].append(sc)

                        cip = pd.tile([128, 3, E], f16, tag="cip")
                        nc.sync.dma_start(out=cip[:nrows], in_=ts["cip"][rsl])
                        cf = cip[:, 0, :]
                        im = cip[:, 1, :]
                        p3 = cip[:, 2, :]  # host-packed 0.3*pca
                        nc.vector.tensor_tensor(
                            out=p3[:nrows], in0=p3[:nrows], in1=amat[:nrows, :],
                            op=mybir.AluOpType.add,
                        )
                        nc.vector.tensor_tensor(
                            out=cf[:nrows], in0=cf[:nrows], in1=im[:nrows],
                            op=mybir.AluOpType.mult,
                        )
                        nc.vector.tensor_tensor(
                            out=p3[:nrows], in0=cf[:nrows], in1=p3[:nrows],
                            op=mybir.AluOpType.mult,
                        )
                        nc.scalar.dma_start(out=ts["out"][rsl], in_=p3[:nrows])

            # phase separation: every local_scatter after every dma_gather so
            # the GPSIMD library (mlp <-> local_scatter) switches once per rep
            if not probe_no_phase_dep:
                for s2 in ("a", "b"):
                    for sc in scatters[s2]:
                        for g in gathers["a"] + gathers["b"]:
                            bass_rust.add_dep_helper(
                                sc.ins, g.ins, False, "library phase separation"
                            )

    if finalize:
        _finalize(nc)
    return nc


class _Runner:
    """Compiles the SPMD bass program through PJRT once and keeps the jitted
    callable for repeated (timed) execution. Mirrors
    concourse.bass2jax.run_bass_via_pjrt, minus output-donation (the kernel
    writes every output element, so uninitialized result buffers are fine)."""

    def __init__(self, nc):
        import jax
        from jax.sharding import Mesh, PartitionSpec
        from jax.experimental.shard_map import shard_map
        from concourse import bass2jax
        import concourse.mybir as _mybir

        bass2jax.install_neuronx_cc_hook()
        self.jax = jax
        partition_name = (
            nc.partition_id_tensor.name if nc.partition_id_tensor else None
        )
        in_names, out_names, out_avals = [], [], []
        for alloc in nc.m.functions[0].allocations:
            if not isinstance(alloc, _mybir.MemoryLocationSet):
                continue
            name = alloc.memorylocations[0].name
            if alloc.kind == "ExternalInput":
                if name != partition_name:
                    in_names.append(name)
            elif alloc.kind == "ExternalOutput":
                out_names.append(name)
                out_avals.append(
                    jax.core.ShapedArray(
                        tuple(alloc.tensor_shape), _mybir.dt.np(alloc.dtype)
                    )
                )
        self.in_names, self.out_names, self.out_avals = in_names, out_names, out_avals

        bind_in_names = tuple(in_names) + (
            (partition_name,) if partition_name else ()
        )

        def _body(*args):
            operands = list(args)
            if partition_name is not None:
                operands.append(bass2jax.partition_id_tensor())
            outs = bass2jax._bass_exec_p.bind(
                *operands,
                out_avals=tuple(out_avals),
                in_names=bind_in_names,
                out_names=tuple(out_names),
                lowering_input_output_aliases=(),
                sim_require_finite=True,
                sim_require_nnan=True,
                nc=nc,
            )
            return tuple(outs)

        devices = jax.devices()[:NCORES]
        self.mesh = Mesh(np.asarray(devices), ("core",))
        in_specs = (PartitionSpec("core"),) * len(in_names)
        out_specs = (PartitionSpec("core"),) * len(out_names)
        self.fn = jax.jit(
            shard_map(
                _body,
                mesh=self.mesh,
                in_specs=in_specs,
                out_specs=out_specs,
                check_rep=False,
            ),
            keep_unused=True,
        )

    def concat_inputs(self, in_maps):
        return [
            np.concatenate([np.asarray(in_maps[c][n]) for c in range(NCORES)], axis=0)
            for n in self.in_names
        ]

    def run(self, concat_in):
        return self.fn(*concat_in)

    def split_outputs(self, out_arrs):
        res = []
        for c in range(NCORES):
            res.append(
                {
                    n: np.asarray(out_arrs[i]).reshape(
                        NCORES, *self.out_avals[i].shape
                    )[c]
                    for i, n in enumerate(self.out_names)
                }
            )
        return res


def _get_runner(M):
    key = ("runner", M)
    if key not in _CACHE:
        nc = _build_nc(M)
        _CACHE[key] = _Runner(nc)
    return _CACHE[key]


def _host_prep(inputs, gw=GW):
    import ml_dtypes

    ent_a = np.asarray(inputs["ent_emb_sr"], np.float32)
    ent_b = np.asarray(inputs["ent_emb_tg"], np.float32)
    rel_a = np.asarray(inputs["rel_emb_sr"], np.float32)
    rel_b = np.asarray(inputs["rel_emb_tg"], np.float32)

    Ms = []
    for sfx in ("sr", "tg"):
        h = np.asarray(inputs[f"head_{sfx}"], np.int64)
        Ms.append(
            max(
                np.bincount(
                    h[(h >= RB * c) & (h < RB * (c + 1))] - RB * c, minlength=RB
                ).max()
                for c in range(NCORES)
            )
        )
    M = int(max(Ms))
    M += M & 1  # local_scatter needs num_idxs % 2 == 0

    in_maps = [dict() for _ in range(NCORES)]
    for s in ("a", "b"):
        sfx = "sr" if s == "a" else "tg"
        ent = ent_a if s == "a" else ent_b
        rel = rel_a if s == "a" else rel_b
        routed = _prep_set(
            ent,
            rel,
            inputs[f"relation_w_{sfx}"],
            inputs[f"head_{sfx}"],
            inputs[f"tail_{sfx}"],
            inputs[f"rel_{sfx}"],
            M,
            gw,
        )
        conf = np.asarray(inputs[f"conf_{sfx}"], np.float32)
        imp = np.asarray(inputs[f"imp_{sfx}"], np.float32)
        pca3 = np.asarray(inputs[f"pca_{sfx}"], np.float32) * np.float32(0.3)
        for c in range(NCORES):
            m = in_maps[c]
            rs = slice(RB * c, RB * (c + 1))
            m[f"cip_{s}"] = np.ascontiguousarray(
                np.stack([conf[rs], imp[rs], pca3[rs]], axis=1)
            ).astype(np.float16)
            m[f"tab_{s}"] = routed[c]["tab"]
            m[f"gix_{s}"] = routed[c]["gix"]
            m[f"lix_{s}"] = routed[c]["lix"]
            m[f"cm_{s}"] = routed[c]["cm"]
    return M, in_maps


def kernel(**inputs):
    M, in_maps = _host_prep(inputs)
    try:
        from concourse._compat import axon_active

        use_pjrt = axon_active()
    except Exception:
        use_pjrt = True
    if use_pjrt:
        runner = _get_runner(M)
        concat_in = runner.concat_inputs(in_maps)
        out_arrs = runner.run(concat_in)
        res = runner.split_outputs(out_arrs)
    else:
        from concourse.bass_utils import run_bass_kernel_spmd

        key = ("nc", M)
        if key not in _CACHE:
            _CACHE[key] = _build_nc(M)
        res = run_bass_kernel_spmd(_CACHE[key], in_maps, list(range(NCORES))).results
    idx = np.arange(E)
    adj_sr = np.concatenate(
        [res[c]["out_a"] for c in range(NCORES)], axis=0
    ).astype(np.float32)
    adj_tg = np.concatenate(
        [res[c]["out_b"] for c in range(NCORES)], axis=0
    ).astype(np.float32)
    adj_sr[idx, idx] += 1.0
    adj_tg[idx, idx] += 1.0
    return adj_sr, adj_tg
